# revision 42
# baseline (speedup 1.0000x reference)
"""Distributed Trainium2 kernel for nn_AdjLayer (conv3x3 -> softmax -> outer(colsum)).

Raw-Bacc implementation (no TileContext) with a manual semaphore pipeline.

  - Host: im2col the tiny input (48400 x 28, incl. ones column for bias) in
    bf16, shard 6050 pixels per core, pad to 6144, pack as [128, 2204] with
    3 pixel-groups at partition offsets 0/32/64 (28 im2col rows each) and
    the weights in the first 156 columns.  Pixels are interleaved across
    tile quadruples (tile 4m+e, matmul column j -> pixel 512m+4j+e) so the
    output DMA writes contiguous 2496-byte DRAM runs.
  - Device (SPMD x8), 16 superblocks of 3 x 128-pixel tiles:
      PE:   3 bf16 matmuls (K=28) per block -> PSUM bank b%8 [128, 468]
      ACT:  one wide Exp per block into a 16-slot exp buffer; normalize of
            tile u2 (Relu x scale == multiply, values >= 0), relu batches
            interleaved between exps to stay dense
      DVE:  per-block 3D row-sum reduce; batched reciprocal (per 4 blocks);
            normalize of tiles u0,u1 (contiguous 2-tile broadcast multiply)
      SYNC: S streamed out f32 in 12 chunked DMAs ([128, 624] each, fully
            contiguous thanks to the quad-interleave)
  - Host: gather S shards (already in pixel order), c = S.sum(0),
    new_adj = outer(c, c).
"""

import sys
from contextlib import ExitStack

import numpy as np

for _p in ("/opt/trn_rl_repo",):
    if _p not in sys.path:
        sys.path.insert(0, _p)

N_F = 156          # filters
N_PIX = 48400      # 220*220 output pixels
N_CORES = 8
PER_CORE = N_PIX // N_CORES   # 6050
K = 28             # 27 conv taps + 1 ones-row (bias)
GROUPS = 3         # pixel groups at partition offsets 0/32/64
TILE = 128         # pixels per matmul tile
TILES_PER_G = 16
G_PIX = TILES_PER_G * TILE        # 2048 pixels per group
PIX_PAD = GROUPS * G_PIX          # 6144 padded pixels per core
N_TILES = GROUPS * TILES_PER_G    # 48
SB = 3                            # tiles per superblock
N_SB = N_TILES // SB              # 16
RB = 4                            # superblocks per reduce/reciprocal batch
CHUNK = 4                         # tiles per output DMA (12 DMAs)
N_CHUNKS = N_TILES // CHUNK
XW_COLS = N_F + G_PIX             # weights cols [0:156], im2col [156:2204]
SPLIT0 = N_F + 4 * TILE           # input dma0 covers W + tiles j<4
SPLIT = N_F + 12 * TILE           # input dma1 covers tiles j<12
N_SLOT = 8                        # psum banks in flight (exp slots = N_SB)

_GRAPH = None


def _build_graph():
    from concourse import bacc
    from concourse import mybir
    from concourse.ap import AP as RawAP

    f32 = mybir.dt.float32
    bf16 = mybir.dt.bfloat16
    nc = bacc.Bacc(None)

    xw_ext = nc.declare_dram_parameter("xw", [128, XW_COLS], bf16, isOutput=False)
    # [48, 128, 156] is byte-identical to [6144, 156] (tile-major rows)
    s_ext = nc.declare_dram_parameter("s", [N_TILES, TILE, N_F], f32, isOutput=True)

    Exp = mybir.ActivationFunctionType.Exp
    Relu = mybir.ActivationFunctionType.Relu
    X = mybir.AxisListType.X

    W_SLOT = SB * N_F  # 468 columns per exp-buffer slot

    # completion bookkeeping: after DVE mul of block b, s_mul == b+1;
    # after ACT relu-scale of block b, s_mula == b+1.  Chunk c needs
    # every tile <= L staged: tiles 3b,3b+1 by DVE, 3b+2 by ACT.
    DVE_THR = []
    ACT_THR = []
    for c in range(N_CHUNKS):
        L = CHUNK * c + CHUNK - 1
        DVE_THR.append(L // SB + 1)
        ACT_THR.append((L - 2) // SB + 1)

    with ExitStack() as ctx:
        xw_sb = ctx.enter_context(nc.sbuf_tensor("xw_sb", [128, XW_COLS], bf16))
        sst = ctx.enter_context(nc.sbuf_tensor("sst", [128, N_TILES * N_F], f32))
        sums_w = ctx.enter_context(nc.sbuf_tensor("sums_w", [128, N_TILES], f32))
        recip_w = ctx.enter_context(nc.sbuf_tensor("recip_w", [128, N_TILES], f32))
        exp_buf = ctx.enter_context(
            nc.sbuf_tensor("exp_buf", [128, N_SB * W_SLOT], f32)
        )
        psum_banks = [
            ctx.enter_context(nc.psum_tensor(f"psbank{i}", [128, W_SLOT], f32))
            for i in range(N_SLOT)
        ]
        s_in = ctx.enter_context(nc.semaphore("s_in"))
        s_in0 = ctx.enter_context(nc.semaphore("s_in0"))
        s_in2 = ctx.enter_context(nc.semaphore("s_in2"))
        s_mm = ctx.enter_context(nc.semaphore("s_mm"))
        s_exp = ctx.enter_context(nc.semaphore("s_exp"))
        s_red = ctx.enter_context(nc.semaphore("s_red"))
        s_rc = ctx.enter_context(nc.semaphore("s_rc"))
        s_mul = ctx.enter_context(nc.semaphore("s_mul"))
        s_mula = ctx.enter_context(nc.semaphore("s_mula"))
        s_out = ctx.enter_context(nc.semaphore("s_out"))

        # issue input DMAs ahead of the Block: they land in the engines'
        # entry streams right after the preamble, overlapping block entry
        nc.sync.dma_start(
            out=xw_sb[:, :SPLIT0], in_=xw_ext[:, :SPLIT0]
        ).then_inc(s_in0, 16)
        nc.sync.dma_start(
            out=xw_sb[:, SPLIT0:SPLIT], in_=xw_ext[:, SPLIT0:SPLIT]
        ).then_inc(s_in, 16)
        nc.scalar.dma_start(
            out=xw_sb[:, SPLIT:], in_=xw_ext[:, SPLIT:]
        ).then_inc(s_in2, 16)

        block = ctx.enter_context(nc.Block(no_gpsimd_drain=True))

        def exp_slot(b):
            s0 = b * W_SLOT
            return exp_buf[:, s0 : s0 + W_SLOT]

        @block.sync
        def _(sync):
            for c in range(N_CHUNKS):
                sync.wait_ge(s_mul, DVE_THR[c])
                sync.wait_ge(s_mula, ACT_THR[c])
                t0 = c * CHUNK
                # pixel(tile 4m+e, part p) == 512m + 4p + e, so one
                # chunk is a contiguous [128, 624] block of DRAM rows
                dst = RawAP(
                    s_ext,
                    t0 * TILE * N_F,
                    [[CHUNK * N_F, 128], [1, CHUNK * N_F]],
                )
                src = sst[:, t0 * N_F : (t0 + CHUNK) * N_F]
                sync.dma_start(out=dst, in_=src).then_inc(s_out, 16)
            sync.wait_ge(s_out, 16 * N_CHUNKS)

        @block.tensor
        def _(tensor):
            tensor.wait_ge(s_in0, 16)
            for b in range(N_SB):
                if b == 1:
                    tensor.wait_ge(s_in, 16)
                if b == 4:
                    tensor.wait_ge(s_in2, 16)
                if b >= N_SLOT:
                    tensor.wait_ge(s_exp, b - N_SLOT + 1)
                pb = psum_banks[b % N_SLOT]
                for u in range(SB):
                    t = SB * b + u
                    g, j = divmod(t, TILES_PER_G)
                    p0 = 32 * g
                    ins = nc.tensor.matmul(
                        pb[:, u * N_F : (u + 1) * N_F],
                        lhsT=xw_sb[
                            p0 : p0 + K, N_F + j * TILE : N_F + (j + 1) * TILE
                        ],
                        rhs=xw_sb[p0 : p0 + K, :N_F],
                        start=True,
                        stop=True,
                    )
                ins.then_inc(s_mm, 1)

        @block.scalar
        def _(scalar):
            def emit_exp(b):
                scalar.wait_ge(s_mm, b + 1)
                nc.scalar.activation(
                    exp_slot(b), psum_banks[b % N_SLOT][:], Exp
                ).then_inc(s_exp, 1)

            def emit_relu_batch(q):
                # first DVE mul of this batch implies the reciprocal
                # completed (ACT->s_rc waits fault at runtime; s_mul works)
                scalar.wait_ge(s_mul, RB * q + 1)
                for bb in range(RB * q, RB * (q + 1)):
                    t2 = SB * bb + 2
                    nc.scalar.activation(
                        sst[:, t2 * N_F : (t2 + 1) * N_F],
                        exp_slot(bb)[:, 2 * N_F : 3 * N_F],
                        Relu,  # x>=0 so Relu(x*scale) == x*scale
                        scale=recip_w[:, t2 : t2 + 1],
                    ).then_inc(s_mula, 1)

            # all exps run dense (the DVE software pipeline needs late
            # exps undelayed so reduces 12-15 aren't starved); relus follow
            for b in range(N_SB):
                emit_exp(b)
            for q in range(N_SB // RB):
                emit_relu_batch(q)

        @block.vector
        def _(vector):
            def emit_red(b):
                vector.wait_ge(s_exp, b + 1)
                nc.vector.reduce_sum(
                    out=sums_w[:, SB * b : SB * (b + 1)],
                    in_=exp_slot(b).rearrange("p (t f) -> p t f", t=SB),
                    axis=X,
                ).then_inc(s_red, 1)

            def emit_recip(q):
                vector.wait_ge(s_red, RB * q + RB)
                nc.vector.reciprocal(
                    recip_w[:, SB * RB * q : SB * RB * (q + 1)],
                    sums_w[:, SB * RB * q : SB * RB * (q + 1)],
                ).then_inc(s_rc, 1)

            def emit_muls(q):
                vector.wait_ge(s_rc, q + 1)
                for bb in range(RB * q, RB * (q + 1)):
                    t0 = SB * bb
                    rec_b = (
                        recip_w[:, t0 : t0 + 2]
                        .rearrange("p (t o) -> p t o", o=1)
                        .broadcast_to([128, 2, N_F])
                    )
                    nc.vector.tensor_mul(
                        sst[:, t0 * N_F : (t0 + 2) * N_F].rearrange(
                            "p (t f) -> p t f", t=2
                        ),
                        exp_slot(bb)[:, : 2 * N_F].rearrange(
                            "p (t f) -> p t f", t=2
                        ),
                        rec_b,
                    ).then_inc(s_mul, 1)

            # software-pipelined: the intervening ops retire the producers
            # each wait needs, so the waits rarely stall the DVE pipeline
            for b in range(8):
                emit_red(b)
            emit_recip(0)
            for b in range(8, 12):
                emit_red(b)
            emit_muls(0)
            emit_recip(1)
            for b in range(12, 16):
                emit_red(b)
            emit_muls(1)
            emit_recip(2)
            emit_muls(2)
            emit_recip(3)
            emit_muls(3)

    nc.finalize()
    return nc


def _get_graph():
    global _GRAPH
    if _GRAPH is None:
        _GRAPH = _build_graph()
    return _GRAPH


def _prepare_inputs(inputs, W, b):
    """Host-side im2col + per-core packing (bf16)."""
    import ml_dtypes
    from numpy.lib.stride_tricks import sliding_window_view

    x = np.ascontiguousarray(np.asarray(inputs, dtype=np.float32)[0])  # [222,222,3]
    W = np.asarray(W, dtype=np.float32)
    b = np.asarray(b, dtype=np.float32)

    # [220,220,3(c),3(dy),3(dx)] -> [y,x,dy,dx,c] -> [48400, 27]
    win = sliding_window_view(x, (3, 3), axis=(0, 1))
    cols = win.transpose(0, 1, 3, 4, 2).reshape(N_PIX, 27)
    cols = np.concatenate(
        [cols, np.ones((N_PIX, 1), dtype=np.float32)], axis=1
    )  # [48400, 28]

    w28 = np.concatenate([W.reshape(27, N_F), b[None, :]], axis=0)  # [28,156]

    in_maps = []
    for i in range(N_CORES):
        shard = cols[i * PER_CORE : (i + 1) * PER_CORE]
        pad = np.zeros((PIX_PAD, K), dtype=np.float32)
        pad[:PER_CORE] = shard
        # tile 4m+e, column j computes core-pixel 512m + 4j + e
        perm = (
            pad.reshape(12, TILE, 4, K).transpose(0, 2, 1, 3).reshape(N_TILES, TILE, K)
        )
        xw = np.zeros((128, XW_COLS), dtype=np.float32)
        for g in range(GROUPS):
            xw[32 * g : 32 * g + K, :N_F] = w28
            xw[32 * g : 32 * g + K, N_F:] = (
                perm[TILES_PER_G * g : TILES_PER_G * (g + 1)]
                .reshape(G_PIX, K)
                .T
            )
        in_maps.append({"xw": xw.astype(ml_dtypes.bfloat16)})
    return in_maps


def _run(inputs, W, b, trace=False):
    from concourse.bass_utils import run_bass_kernel_spmd

    in_maps = _prepare_inputs(inputs, W, b)
    nc = _get_graph()
    res = run_bass_kernel_spmd(
        nc, in_maps, core_ids=list(range(N_CORES)), trace=trace
    )

    S = np.empty((N_PIX, N_F), dtype=np.float32)
    for i in range(N_CORES):
        # quad-interleaved staging lands in natural pixel order in DRAM
        S[i * PER_CORE : (i + 1) * PER_CORE] = res.results[i]["s"].reshape(
            PIX_PAD, N_F
        )[:PER_CORE]
    c = S.sum(axis=0, dtype=np.float64).astype(np.float32)
    new_adj = np.outer(c, c).astype(np.float32)
    return (new_adj, S), res


def kernel(**inputs):
    (new_adj, S), _ = _run(inputs["inputs"], inputs["W"], inputs["b"])
    return (new_adj, S)


# revision 43
# speedup vs baseline: 1.2201x; 1.2201x over previous
"""Distributed Trainium2 kernel for nn_AdjLayer (conv3x3 -> softmax -> outer(colsum)).

Raw-Bacc implementation (no TileContext) with a manual semaphore pipeline.

  - Host: im2col the tiny input (48400 x 28, incl. ones column for bias) in
    bf16, shard 6050 pixels per core, pad to 6144, pack as [128, 2204] with
    3 pixel-groups at partition offsets 0/32/64 (28 im2col rows each) and
    the weights in the first 156 columns.  Pixels are interleaved across
    tile quadruples (tile 4m+e, matmul column j -> pixel 512m+4j+e) so the
    output DMA writes contiguous 2496-byte DRAM runs.
  - Device (SPMD x8), 16 superblocks of 3 x 128-pixel tiles:
      PE:   3 bf16 matmuls (K=28) per block -> PSUM bank b%8 [128, 468]
      ACT:  one wide Exp per block into a 16-slot exp buffer; normalize of
            tile u2 (Relu x scale == multiply, values >= 0), relu batches
            interleaved between exps to stay dense
      DVE:  per-block 3D row-sum reduce; batched reciprocal (per 4 blocks);
            normalize of tiles u0,u1 (contiguous 2-tile broadcast multiply)
      SYNC: S streamed out f32 in 12 chunked DMAs ([128, 624] each, fully
            contiguous thanks to the quad-interleave)
  - Host: gather S shards (already in pixel order), c = S.sum(0),
    new_adj = outer(c, c).
"""

import sys
from contextlib import ExitStack

import numpy as np

for _p in ("/opt/trn_rl_repo",):
    if _p not in sys.path:
        sys.path.insert(0, _p)

N_F = 156          # filters
N_PIX = 48400      # 220*220 output pixels
N_CORES = 8
PER_CORE = N_PIX // N_CORES   # 6050
K = 28             # 27 conv taps + 1 ones-row (bias)
GROUPS = 3         # pixel groups at partition offsets 0/32/64
TILE = 128         # pixels per matmul tile
TILES_PER_G = 16
G_PIX = TILES_PER_G * TILE        # 2048 pixels per group
PIX_PAD = GROUPS * G_PIX          # 6144 padded pixels per core
N_TILES = GROUPS * TILES_PER_G    # 48
SB = 3                            # tiles per superblock
N_SB = N_TILES // SB              # 16
RB = 4                            # superblocks per reduce/reciprocal batch
CHUNK = 4                         # tiles per output DMA (12 DMAs)
N_CHUNKS = N_TILES // CHUNK
XW_COLS = N_F + G_PIX             # weights cols [0:156], im2col [156:2204]
SPLIT0 = N_F + 4 * TILE           # input dma0 covers W + tiles j<4
SPLIT = N_F + 12 * TILE           # input dma1 covers tiles j<12
N_SLOT = 8                        # psum banks in flight (exp slots = N_SB)

_GRAPH = None


def _build_graph():
    from concourse import bacc
    from concourse import mybir
    from concourse.ap import AP as RawAP

    f32 = mybir.dt.float32
    bf16 = mybir.dt.bfloat16
    nc = bacc.Bacc(None)

    xw_ext = nc.declare_dram_parameter("xw", [128, XW_COLS], bf16, isOutput=False)
    # [48, 128, 156] is byte-identical to [6144, 156] (tile-major rows)
    s_ext = nc.declare_dram_parameter("s", [N_TILES, TILE, N_F], f32, isOutput=True)

    Exp = mybir.ActivationFunctionType.Exp
    Relu = mybir.ActivationFunctionType.Relu
    X = mybir.AxisListType.X

    W_SLOT = SB * N_F  # 468 columns per exp-buffer slot

    # completion bookkeeping: after DVE mul of block b, s_mul == b+1;
    # after ACT relu-scale of block b, s_mula == b+1.  Chunk c needs
    # every tile <= L staged: tiles 3b,3b+1 by DVE, 3b+2 by ACT.
    DVE_THR = []
    ACT_THR = []
    for c in range(N_CHUNKS):
        L = CHUNK * c + CHUNK - 1
        DVE_THR.append(L // SB + 1)
        ACT_THR.append((L - 2) // SB + 1)

    with ExitStack() as ctx:
        xw_sb = ctx.enter_context(nc.sbuf_tensor("xw_sb", [128, XW_COLS], bf16))
        sst = ctx.enter_context(nc.sbuf_tensor("sst", [128, N_TILES * N_F], f32))
        sums_w = ctx.enter_context(nc.sbuf_tensor("sums_w", [128, N_TILES], f32))
        recip_w = ctx.enter_context(nc.sbuf_tensor("recip_w", [128, N_TILES], f32))
        exp_buf = ctx.enter_context(
            nc.sbuf_tensor("exp_buf", [128, N_SB * W_SLOT], f32)
        )
        psum_banks = [
            ctx.enter_context(nc.psum_tensor(f"psbank{i}", [128, W_SLOT], f32))
            for i in range(N_SLOT)
        ]
        s_in = ctx.enter_context(nc.semaphore("s_in"))
        s_in0 = ctx.enter_context(nc.semaphore("s_in0"))
        s_in2 = ctx.enter_context(nc.semaphore("s_in2"))
        s_mm = ctx.enter_context(nc.semaphore("s_mm"))
        s_exp = ctx.enter_context(nc.semaphore("s_exp"))
        s_red = ctx.enter_context(nc.semaphore("s_red"))
        s_rc = ctx.enter_context(nc.semaphore("s_rc"))
        s_mul = ctx.enter_context(nc.semaphore("s_mul"))
        s_mula = ctx.enter_context(nc.semaphore("s_mula"))
        s_out = ctx.enter_context(nc.semaphore("s_out"))

        # issue input DMAs ahead of the Block: they land in the engines'
        # entry streams right after the preamble, overlapping block entry
        nc.sync.dma_start(
            out=xw_sb[:, :SPLIT0], in_=xw_ext[:, :SPLIT0]
        ).then_inc(s_in0, 16)
        nc.sync.dma_start(
            out=xw_sb[:, SPLIT0:SPLIT], in_=xw_ext[:, SPLIT0:SPLIT]
        ).then_inc(s_in, 16)
        nc.scalar.dma_start(
            out=xw_sb[:, SPLIT:], in_=xw_ext[:, SPLIT:]
        ).then_inc(s_in2, 16)

        block = ctx.enter_context(nc.Block(no_gpsimd_drain=True))

        def exp_slot(b):
            s0 = b * W_SLOT
            return exp_buf[:, s0 : s0 + W_SLOT]

        @block.sync
        def _(sync):
            for c in range(N_CHUNKS):
                sync.wait_ge(s_mul, DVE_THR[c])
                sync.wait_ge(s_mula, ACT_THR[c])
                t0 = c * CHUNK
                # pixel(tile 4m+e, part p) == 512m + 4p + e, so one
                # chunk is a contiguous [128, 624] block of DRAM rows
                dst = RawAP(
                    s_ext,
                    t0 * TILE * N_F,
                    [[CHUNK * N_F, 128], [1, CHUNK * N_F]],
                )
                src = sst[:, t0 * N_F : (t0 + CHUNK) * N_F]
                sync.dma_start(out=dst, in_=src).then_inc(s_out, 16)
            # no final s_out wait: the SP DRAIN at block end covers HWDGE
            # completion (same mechanism TileContext kernels rely on),
            # skipping the ~2us semaphore-receipt latency of the last chunk

        @block.tensor
        def _(tensor):
            tensor.wait_ge(s_in0, 16)
            for b in range(N_SB):
                if b == 1:
                    tensor.wait_ge(s_in, 16)
                if b == 4:
                    tensor.wait_ge(s_in2, 16)
                if b >= N_SLOT:
                    tensor.wait_ge(s_exp, b - N_SLOT + 1)
                pb = psum_banks[b % N_SLOT]
                for u in range(SB):
                    t = SB * b + u
                    g, j = divmod(t, TILES_PER_G)
                    p0 = 32 * g
                    ins = nc.tensor.matmul(
                        pb[:, u * N_F : (u + 1) * N_F],
                        lhsT=xw_sb[
                            p0 : p0 + K, N_F + j * TILE : N_F + (j + 1) * TILE
                        ],
                        rhs=xw_sb[p0 : p0 + K, :N_F],
                        start=True,
                        stop=True,
                    )
                ins.then_inc(s_mm, 1)

        @block.scalar
        def _(scalar):
            def emit_exp(b):
                scalar.wait_ge(s_mm, b + 1)
                nc.scalar.activation(
                    exp_slot(b), psum_banks[b % N_SLOT][:], Exp
                ).then_inc(s_exp, 1)

            def emit_relu_batch(q):
                # first DVE mul of this batch implies the reciprocal
                # completed (ACT->s_rc waits fault at runtime; s_mul works)
                scalar.wait_ge(s_mul, RB * q + 1)
                for bb in range(RB * q, RB * (q + 1)):
                    t2 = SB * bb + 2
                    nc.scalar.activation(
                        sst[:, t2 * N_F : (t2 + 1) * N_F],
                        exp_slot(bb)[:, 2 * N_F : 3 * N_F],
                        Relu,  # x>=0 so Relu(x*scale) == x*scale
                        scale=recip_w[:, t2 : t2 + 1],
                    ).then_inc(s_mula, 1)

            # all exps run dense (the DVE software pipeline needs late
            # exps undelayed so reduces 12-15 aren't starved); relus follow
            for b in range(N_SB):
                emit_exp(b)
            for q in range(N_SB // RB):
                emit_relu_batch(q)

        @block.vector
        def _(vector):
            def emit_red(b):
                vector.wait_ge(s_exp, b + 1)
                nc.vector.reduce_sum(
                    out=sums_w[:, SB * b : SB * (b + 1)],
                    in_=exp_slot(b).rearrange("p (t f) -> p t f", t=SB),
                    axis=X,
                ).then_inc(s_red, 1)

            def emit_recip(q):
                vector.wait_ge(s_red, RB * q + RB)
                nc.vector.reciprocal(
                    recip_w[:, SB * RB * q : SB * RB * (q + 1)],
                    sums_w[:, SB * RB * q : SB * RB * (q + 1)],
                ).then_inc(s_rc, 1)

            def emit_muls(q):
                vector.wait_ge(s_rc, q + 1)
                for bb in range(RB * q, RB * (q + 1)):
                    t0 = SB * bb
                    rec_b = (
                        recip_w[:, t0 : t0 + 2]
                        .rearrange("p (t o) -> p t o", o=1)
                        .broadcast_to([128, 2, N_F])
                    )
                    nc.vector.tensor_mul(
                        sst[:, t0 * N_F : (t0 + 2) * N_F].rearrange(
                            "p (t f) -> p t f", t=2
                        ),
                        exp_slot(bb)[:, : 2 * N_F].rearrange(
                            "p (t f) -> p t f", t=2
                        ),
                        rec_b,
                    ).then_inc(s_mul, 1)

            # software-pipelined: the intervening ops retire the producers
            # each wait needs, so the waits rarely stall the DVE pipeline
            for b in range(8):
                emit_red(b)
            emit_recip(0)
            for b in range(8, 12):
                emit_red(b)
            emit_muls(0)
            emit_recip(1)
            for b in range(12, 16):
                emit_red(b)
            emit_muls(1)
            emit_recip(2)
            emit_muls(2)
            emit_recip(3)
            emit_muls(3)

    nc.finalize()
    return nc


def _get_graph():
    global _GRAPH
    if _GRAPH is None:
        _GRAPH = _build_graph()
    return _GRAPH


def _prepare_inputs(inputs, W, b):
    """Host-side im2col + per-core packing (bf16)."""
    import ml_dtypes
    from numpy.lib.stride_tricks import sliding_window_view

    x = np.ascontiguousarray(np.asarray(inputs, dtype=np.float32)[0])  # [222,222,3]
    W = np.asarray(W, dtype=np.float32)
    b = np.asarray(b, dtype=np.float32)

    # [220,220,3(c),3(dy),3(dx)] -> [y,x,dy,dx,c] -> [48400, 27]
    win = sliding_window_view(x, (3, 3), axis=(0, 1))
    cols = win.transpose(0, 1, 3, 4, 2).reshape(N_PIX, 27)
    cols = np.concatenate(
        [cols, np.ones((N_PIX, 1), dtype=np.float32)], axis=1
    )  # [48400, 28]

    w28 = np.concatenate([W.reshape(27, N_F), b[None, :]], axis=0)  # [28,156]

    in_maps = []
    for i in range(N_CORES):
        shard = cols[i * PER_CORE : (i + 1) * PER_CORE]
        pad = np.zeros((PIX_PAD, K), dtype=np.float32)
        pad[:PER_CORE] = shard
        # tile 4m+e, column j computes core-pixel 512m + 4j + e
        perm = (
            pad.reshape(12, TILE, 4, K).transpose(0, 2, 1, 3).reshape(N_TILES, TILE, K)
        )
        xw = np.zeros((128, XW_COLS), dtype=np.float32)
        for g in range(GROUPS):
            xw[32 * g : 32 * g + K, :N_F] = w28
            xw[32 * g : 32 * g + K, N_F:] = (
                perm[TILES_PER_G * g : TILES_PER_G * (g + 1)]
                .reshape(G_PIX, K)
                .T
            )
        in_maps.append({"xw": xw.astype(ml_dtypes.bfloat16)})
    return in_maps


def _run(inputs, W, b, trace=False):
    from concourse.bass_utils import run_bass_kernel_spmd

    in_maps = _prepare_inputs(inputs, W, b)
    nc = _get_graph()
    res = run_bass_kernel_spmd(
        nc, in_maps, core_ids=list(range(N_CORES)), trace=trace
    )

    S = np.empty((N_PIX, N_F), dtype=np.float32)
    for i in range(N_CORES):
        # quad-interleaved staging lands in natural pixel order in DRAM
        S[i * PER_CORE : (i + 1) * PER_CORE] = res.results[i]["s"].reshape(
            PIX_PAD, N_F
        )[:PER_CORE]
    c = S.sum(axis=0, dtype=np.float64).astype(np.float32)
    new_adj = np.outer(c, c).astype(np.float32)
    return (new_adj, S), res


def kernel(**inputs):
    (new_adj, S), _ = _run(inputs["inputs"], inputs["W"], inputs["b"])
    return (new_adj, S)
